# revision 18
# baseline (speedup 1.0000x reference)
"""BipartiteGConv Trainium2 kernel (8 NeuronCores, node sharding).

Math (see reference):
  rhs = input @ Wi + bi            [N_IN, D]
  lhs = other @ Wo                 [N_OT, D]
  msg = lrelu(rhs[rj] + lhs[lj] + w*We) per edge
  S   = segment_sum(msg, rj)       [N_IN, D]
  out = concat([S @ Wf + bf, input]) @ Wout + bout
      = S @ (Wf@W1) + counts x (bf@W1) + input @ W2 + bout

Sharding: nodes (rj ranges of 12500) across 8 cores; each core owns all
edges targeting its range.  Edge slots ordered by (lj-segment, rj-window
of 127 nodes), padded per (window, seg) to uniform tile counts across
cores (SPMD).

Per 128-token tile:
  - DVE builds ohw [128 tok, 128]: cols 0..126 onehot of window-local rj,
    col 127 = edge weight.
  - PE transposes it (ohwT) -> ACT copies to SBUF.
  - PE matmul  ohwT.T @ [rhs_win;We]  = rhs[rj] + w*We  (rhs table RW is
    built on-chip and resident in SBUF -- no rhs gather at all).
  - lhs rows come from one dma_gather (bf16 256B rows) per 4096-token
    block out of a DRAM table built on-chip.
  - DVE adds psum+gathered, ACT lrelu.
  - PE scatter matmul lhsT=ohw[:,0:127] accumulates S per window in PSUM.
"""
import sys
sys.path.insert(0, "/opt/trn_rl_repo")
import numpy as np
import ml_dtypes

N_IN, N_OT, E, D = 100000, 50000, 1000000, 64
NC = 8
NPC = N_IN // NC            # nodes per core
W = 127                     # window size (nodes); row 127 = weight slot
NW = (NPC + W - 1) // W     # windows per core (99)
SEG_SPLIT = 32768           # lhs table A/B split (int16 idx limit)
BLK = 1024                  # gather tokens per instruction (SWDGE ring limit)
TPB = BLK // 128            # tiles per block (8)
PADV = 999.0                # rjl value for pad slots (onehot -> 0)


def _wrap16(a):
    # token i -> [i % 16, i // 16], replicated to 128 partitions
    n = a.shape[0]
    assert n % 16 == 0
    return np.tile(a.reshape(n // 16, 16).T, (8, 1)).copy()


def kernel(input, other, rj, lj, weights, Wi, bi, Wo, We, Wf, bf, Wout, bout):
    import concourse.bass as bass
    import concourse.bacc as bacc
    import concourse.mybir as mybir
    import concourse.tile as tile
    from concourse.bass_utils import run_bass_kernel_spmd
    from contextlib import ExitStack

    input = np.asarray(input, np.float32)
    other = np.asarray(other, np.float32)
    rj = np.asarray(rj).astype(np.int64)
    lj = np.asarray(lj).astype(np.int64)
    weights = np.asarray(weights, np.float32).reshape(-1)
    Wi = np.asarray(Wi, np.float32); bi = np.asarray(bi, np.float32)
    Wo = np.asarray(Wo, np.float32); We = np.asarray(We, np.float32).reshape(-1)
    Wf = np.asarray(Wf, np.float32); bf = np.asarray(bf, np.float32)
    Wout = np.asarray(Wout, np.float32); bout = np.asarray(bout, np.float32)

    bf16 = ml_dtypes.bfloat16

    # ---------------- host index prep (per core) ----------------
    core_of = rj // NPC
    order0 = np.argsort(core_of, kind="stable")
    tiles_per = np.zeros((NC, 2, NW), np.int64)
    core_data = []
    for c in range(NC):
        sel = order0[np.searchsorted(core_of, c, side="left", sorter=order0):
                     np.searchsorted(core_of, c, side="right", sorter=order0)]
        rjl_all = rj[sel] - c * NPC
        win = rjl_all // W
        seg = (lj[sel] >= SEG_SPLIT).astype(np.int64)
        key = seg * NW + win
        o2 = np.argsort(key, kind="stable")
        core_data.append((sel[o2], (rjl_all % W)[o2], key[o2]))
        cnt = np.bincount(key[o2], minlength=2 * NW).reshape(2, NW)
        tiles_per[c] = (cnt + 127) // 128
    TW = tiles_per.max(axis=0)          # uniform tiles per (seg, window)
    TA = int(TW[0].sum()); TB = int(TW[1].sum())
    padA = (-TA) % TPB
    padB = (-(TA + padA + TB)) % TPB
    sched = []
    for w in range(NW):
        sched += [(0, w)] * int(TW[0, w])
    sched += [(0, -1)] * padA
    for w in range(NW):
        sched += [(1, w)] * int(TW[1, w])
    sched += [(1, -1)] * padB
    T = len(sched)
    S = T * 128
    NBLK = T // TPB
    last_of = {}                        # (seg, w) -> last tile index
    for t, (sg, w) in enumerate(sched):
        if w >= 0:
            last_of[(sg, w)] = t
    TA_tok = (TA + padA) * 128          # segment A token count (BLK-aligned)

    lhs_idx = np.zeros((NC, S), np.int16)
    rjl_grid = np.full((NC, S), PADV, np.float32)
    w_grid = np.zeros((NC, S), np.float32)
    counts = np.zeros((NC, NPC), np.float32)
    pos_of_group = {}
    p = 0
    for (sg, w) in sched:
        if w >= 0 and (sg, w) not in pos_of_group:
            pos_of_group[(sg, w)] = p
        p += 128
    for c in range(NC):
        sel, rjl_loc, key = core_data[c]
        counts[c] = np.bincount(rj[sel] - c * NPC, minlength=NPC)
        for sg in range(2):
            for w in range(NW):
                k = sg * NW + w
                lo = np.searchsorted(key, k, side="left")
                hi = np.searchsorted(key, k, side="right")
                if lo == hi:
                    continue
                base = pos_of_group[(sg, w)]
                idxs = np.arange(base, base + (hi - lo))
                ee = sel[lo:hi]
                lv = lj[ee] - sg * SEG_SPLIT
                lhs_idx[c, idxs] = lv.astype(np.int16)
                rjl_grid[c, idxs] = rjl_loc[lo:hi].astype(np.float32)
                w_grid[c, idxs] = weights[ee]

    # slot s maps to (p, t) = (s % 128, s // 128)
    def grid_pt(a, dt):
        return np.ascontiguousarray(a.reshape(T, 128).T).astype(dt)

    # ---------------- build bass kernel ----------------
    dt = mybir.dt
    nc = bacc.Bacc("TRN2", target_bir_lowering=False, debug=False,
                   num_devices=NC, num_swdge_queues=4)

    inT_ext = nc.dram_tensor("inT", [65, NPC], dt.bfloat16, kind="ExternalInput").ap()
    otT_ext = nc.dram_tensor("otT", [64, N_OT], dt.bfloat16, kind="ExternalInput").ap()
    WiB_ext = nc.dram_tensor("WiB", [65, 64], dt.bfloat16, kind="ExternalInput").ap()
    Wo_ext = nc.dram_tensor("Wo_", [64, 64], dt.bfloat16, kind="ExternalInput").ap()
    M1_ext = nc.dram_tensor("M1_", [64, 64], dt.bfloat16, kind="ExternalInput").ap()
    W2_ext = nc.dram_tensor("W2_", [64, 64], dt.bfloat16, kind="ExternalInput").ap()
    vb_ext = nc.dram_tensor("vb_", [2, 64], dt.bfloat16, kind="ExternalInput").ap()
    cnts_ext = nc.dram_tensor("cnts", [2, NPC], dt.bfloat16, kind="ExternalInput").ap()
    WeR_ext = nc.dram_tensor("WeR", [1, NW * 64], dt.bfloat16, kind="ExternalInput").ap()
    iota_ext = nc.dram_tensor("iot", [128, 128], dt.bfloat16, kind="ExternalInput").ap()
    lix_ext = nc.dram_tensor("lix", [128, S // 16], dt.int16, kind="ExternalInput").ap()
    rjl_ext = nc.dram_tensor("rjl", [128, T], dt.bfloat16, kind="ExternalInput").ap()
    wg_ext = nc.dram_tensor("wg", [128, T], dt.bfloat16, kind="ExternalInput").ap()
    y_ext = nc.dram_tensor("y", [NPC, 64], dt.float32, kind="ExternalOutput").ap()

    ltabA = nc.dram_tensor("ltabA", [SEG_SPLIT, 128], dt.bfloat16).ap()
    ltabB = nc.dram_tensor("ltabB", [N_OT - SEG_SPLIT, 128], dt.bfloat16).ap()

    with tile.TileContext(nc) as tc, ExitStack() as ctx:
        cpool = ctx.enter_context(tc.tile_pool(name="const", bufs=1))
        tabp = ctx.enter_context(tc.tile_pool(name="tab", bufs=3))
        gp = ctx.enter_context(tc.tile_pool(name="gath", bufs=2))
        ohp = ctx.enter_context(tc.tile_pool(name="ohp", bufs=2))
        ohtp = ctx.enter_context(tc.tile_pool(name="ohtp", bufs=2))
        mp = ctx.enter_context(tc.tile_pool(name="mp", bufs=2))
        wk = ctx.enter_context(tc.tile_pool(name="work", bufs=4))
        psA = ctx.enter_context(tc.tile_pool(name="psA", bufs=2, space="PSUM"))
        accp = ctx.enter_context(tc.tile_pool(name="acc", bufs=1))

        iota = cpool.tile([128, 128], dt.bfloat16)
        nc.sync.dma_start(out=iota[:], in_=iota_ext[:])
        WiB = cpool.tile([65, 64], dt.bfloat16)
        nc.sync.dma_start(out=WiB[:], in_=WiB_ext[:])
        Wo_t = cpool.tile([64, 64], dt.bfloat16)
        nc.sync.dma_start(out=Wo_t[:], in_=Wo_ext[:])
        rjl = cpool.tile([128, T], dt.bfloat16)
        nc.sync.dma_start(out=rjl[:], in_=rjl_ext[:])
        wg = cpool.tile([128, T], dt.bfloat16)
        nc.sync.dma_start(out=wg[:], in_=wg_ext[:])
        lix = cpool.tile([128, S // 16], dt.int16)
        nc.sync.dma_start(out=lix[:], in_=lix_ext[:])
        inT = cpool.tile([65, NPC], dt.bfloat16)
        nc.sync.dma_start(out=inT[:], in_=inT_ext[:])
        from concourse.masks import make_identity
        ident = cpool.tile([128, 128], dt.bfloat16)
        make_identity(nc, ident[:])

        acc = accp.tile([128, NW, 64], dt.float32)
        nc.vector.memset(acc[:], 0.0)

        # ---- resident rhs table RW [128, NW, 64]:
        #      rows 0..126 = rhs nodes of window, row 127 = We ----
        RW = cpool.tile([128, NW, 64], dt.bfloat16)
        nc.vector.memset(RW[:], 0.0)
        # row 127 of every window = We (DMA: engines can't address p127 alone)
        nc.sync.dma_start(out=RW[127:128, :, :], in_=WeR_ext[:])
        for w0 in range(0, NW, 8):
            wn = min(8, NW - w0)
            ps = psA.tile([128, 512], dt.float32, tag="bps", name=f"psrw{w0}")
            for k in range(wn):
                n0 = (w0 + k) * W
                m = min(W, NPC - n0)
                nc.tensor.matmul(out=ps[:m, k * 64:(k + 1) * 64],
                                 lhsT=inT[:, n0:n0 + m],
                                 rhs=WiB[:], start=True, stop=True)
                # copy only valid node rows (stale psum rows stay zero in RW)
                nc.scalar.copy(out=RW[0:m, w0 + k, :],
                               in_=ps[:m, k * 64:(k + 1) * 64])

        # ---- lhs table build: big input DMAs, 8 matmuls per PSUM bank ----
        def build_tab(dst, src_ext, n_rows, wmat, roff=0, CH=2048):
            done = 0
            while done < n_rows:
                todo = min(CH, n_rows - done)
                srct = tabp.tile([64, CH], dt.bfloat16, tag="bsrc")
                nc.sync.dma_start(out=srct[:, :todo],
                                  in_=src_ext[:, roff + done:roff + done + todo])
                ngr = (todo + 127) // 128
                if todo < ngr * 128:
                    nc.vector.memset(srct[:, todo:ngr * 128], 0.0)
                for k0 in range(0, ngr, 8):
                    kn = min(8, ngr - k0)
                    ps = psA.tile([128, 512], dt.float32, tag="bps")
                    for k in range(kn):
                        r0 = (k0 + k) * 128
                        nc.tensor.matmul(out=ps[:, k * 64:(k + 1) * 64],
                                         lhsT=srct[:, r0:r0 + 128],
                                         rhs=wmat[:], start=True, stop=True)
                    sb = tabp.tile([128, 8, 64], dt.bfloat16, tag="bsb")
                    nc.scalar.copy(
                        out=sb[:, :kn, :].rearrange("p k d -> p (k d)"),
                        in_=ps[:, :kn * 64])
                    for k in range(kn):
                        rr = done + (k0 + k) * 128
                        mm = min(128, n_rows - rr)
                        nc.sync.dma_start(out=dst[rr:rr + mm, 0:64],
                                          in_=sb[:mm, k, :])
                done += todo

        build_tab(ltabA, otT_ext, SEG_SPLIT, Wo_t)
        build_tab(ltabB, otT_ext, N_OT - SEG_SPLIT, Wo_t, roff=SEG_SPLIT)

        # ---- main loop ----
        cur = {"w": None, "seg": None, "ps": None, "first": True}

        def flush():
            # psW row 127 accumulates w-column junk; acc row 127 is never
            # read downstream (final phase only uses nodes 0..126)
            if cur["ps"] is not None:
                wv = cur["w"]
                nc.vector.tensor_tensor(out=acc[:, wv, :], in0=acc[:, wv, :],
                                        in1=cur["ps"][:, :], op=mybir.AluOpType.add)
                cur["ps"] = None

        mctx = ExitStack()
        psTM = mctx.enter_context(tc.tile_pool(name="psTM", bufs=2, space="PSUM"))
        psWp = mctx.enter_context(tc.tile_pool(name="psW", bufs=2, space="PSUM"))
        for b in range(NBLK):
            t0 = b * TPB
            tab_ap = ltabA[:, :] if (b * BLK) < TA_tok else ltabB[:, :]
            gl = gp.tile([128, TPB, 128], dt.bfloat16, tag="gl")
            nc.gpsimd.dma_gather(gl[:], tab_ap,
                                 lix[:, b * (BLK // 16):(b + 1) * (BLK // 16)],
                                 BLK, BLK, 128, queue_num=b % 4)
            # ohw: cols 0..126 onehot of rjl, col 127 = edge weight
            ohw = ohp.tile([128, TPB, 128], dt.bfloat16, tag="ohw")
            nc.vector.tensor_tensor(
                out=ohw[:, :, 0:127],
                in0=iota[:, None, 0:127].to_broadcast([128, TPB, 127]),
                in1=rjl[:, t0:t0 + TPB, None].to_broadcast([128, TPB, 127]),
                op=mybir.AluOpType.is_equal)
            nc.vector.tensor_copy(out=ohw[:, :, 127], in_=wg[:, t0:t0 + TPB])

            mrb = mp.tile([128, TPB, 64], dt.bfloat16, tag="mrb")
            for g in range(TPB // 4):
                pst = psTM.tile([128, 4, 128], dt.bfloat16, tag="pst")
                for j in range(4):
                    i = g * 4 + j
                    nc.tensor.transpose(out=pst[:, j, :], in_=ohw[:, i, :],
                                        identity=ident[:])
                ohT = ohtp.tile([128, 4, 128], dt.bfloat16, tag="ohT")
                nc.scalar.copy(out=ohT[:], in_=pst[:])
                psm = psTM.tile([128, 4, 64], dt.float32, tag="psm")
                for j in range(4):
                    i = g * 4 + j
                    sg, wv = sched[t0 + i]
                    wv_ = max(wv, 0)
                    nc.tensor.matmul(out=psm[:, j, :], lhsT=ohT[:, j, :],
                                     rhs=RW[:, wv_, :], start=True, stop=True)
                mpre = wk.tile([128, 4, 64], dt.bfloat16, tag="mpre")
                nc.vector.tensor_tensor(out=mpre[:], in0=psm[:],
                                        in1=gl[:, g * 4:g * 4 + 4, 0:64],
                                        op=mybir.AluOpType.add)
                nc.scalar.activation(out=mrb[:, g * 4:g * 4 + 4, :], in_=mpre[:],
                                     func=mybir.ActivationFunctionType.Lrelu,
                                     alpha=0.01)
            for i in range(TPB):
                t = t0 + i
                sg, wv = sched[t]
                if wv < 0:
                    continue
                if cur["w"] != wv or cur["seg"] != sg:
                    flush()
                    cur["w"], cur["seg"] = wv, sg
                    cur["ps"] = psWp.tile([128, 64], dt.float32, tag="psw",
                                          name=f"psw{t}")
                    cur["first"] = True
                nc.tensor.matmul(out=cur["ps"][:, :], lhsT=ohw[:, i, :],
                                 rhs=mrb[:, i, :],
                                 start=cur["first"],
                                 stop=(t == last_of[(sg, wv)]))
                cur["first"] = False
        flush()
        mctx.close()

        # ---- final: y = S@M1 + counts x v1 + 1 x bout + input@W2 ----
        psF = ctx.enter_context(tc.tile_pool(name="psF", bufs=3, space="PSUM"))
        M1t = cpool.tile([64, 64], dt.bfloat16)
        nc.sync.dma_start(out=M1t[:], in_=M1_ext[:])
        W2t = cpool.tile([64, 64], dt.bfloat16)
        nc.sync.dma_start(out=W2t[:], in_=W2_ext[:])
        vbt = cpool.tile([2, 64], dt.bfloat16)
        nc.sync.dma_start(out=vbt[:], in_=vb_ext[:])
        cntr = cpool.tile([2, NPC], dt.bfloat16)
        nc.sync.dma_start(out=cntr[:], in_=cnts_ext[:])

        for w in range(NW):
            n0 = w * W
            n1 = min(NPC, n0 + W)
            m = n1 - n0
            swb = wk.tile([128, 64], dt.bfloat16, tag="swb")
            nc.scalar.copy(out=swb[:], in_=acc[:, w, :])
            tps = psF.tile([128, 128], dt.bfloat16, tag="fps")
            nc.tensor.transpose(out=tps[0:64, :], in_=swb[:], identity=ident[:])
            swT = wk.tile([64, 128], dt.bfloat16, tag="swT")
            nc.scalar.copy(out=swT[:], in_=tps[0:64, :])
            ops = psF.tile([128, 64], dt.float32, tag="ops")
            nc.tensor.matmul(out=ops[:m, :], lhsT=swT[:, :m], rhs=M1t[:],
                             start=True, stop=False)
            nc.tensor.matmul(out=ops[:m, :], lhsT=inT[0:64, n0:n1], rhs=W2t[:],
                             start=False, stop=False)
            nc.tensor.matmul(out=ops[:m, :], lhsT=cntr[:, n0:n1], rhs=vbt[:],
                             start=False, stop=True)
            ob = wk.tile([128, 64], dt.float32, tag="ob")
            nc.scalar.copy(out=ob[:m, :], in_=ops[:m, :])
            nc.sync.dma_start(out=y_ext[n0:n1, :], in_=ob[:m, :])

    nc.compile()

    # ---------------- host-side in_maps ----------------
    W1 = Wout[:64]; W2 = Wout[64:]
    M1 = (Wf @ W1).astype(np.float32)
    v1 = (bf @ W1).astype(np.float32)
    vb = np.stack([v1, bout]).astype(bf16)
    iota_np = np.tile(np.arange(128, dtype=np.float32)[None, :], (128, 1)).astype(bf16)
    WiB_np = np.concatenate([Wi, bi[None, :]], 0).astype(bf16)
    otT_np = np.ascontiguousarray(other.T).astype(bf16)

    in_maps = []
    for c in range(NC):
        sl = input[c * NPC:(c + 1) * NPC]
        inT_np = np.concatenate([sl.T, np.ones((1, NPC), np.float32)], 0).astype(bf16)
        in_maps.append({
            "inT": np.ascontiguousarray(inT_np),
            "otT": otT_np,
            "WiB": WiB_np, "Wo_": Wo.astype(bf16),
            "M1_": M1.astype(bf16), "W2_": W2.astype(bf16), "vb_": vb,
            "cnts": np.stack([counts[c], np.ones(NPC, np.float32)]).astype(bf16),
            "WeR": np.tile(We.astype(bf16), NW)[None, :], "iot": iota_np,
            "lix": _wrap16(lhs_idx[c]),
            "rjl": grid_pt(rjl_grid[c], bf16),
            "wg": grid_pt(w_grid[c], bf16),
        })

    import os
    res = run_bass_kernel_spmd(nc, in_maps, list(range(NC)),
                               trace=bool(os.environ.get("KTRACE")))
    if os.environ.get("KTRACE") and res.exec_time_ns:
        print(f"HW exec time: {res.exec_time_ns} ns")
    out = np.concatenate([res.results[c]["y"] for c in range(NC)], 0)
    return out.astype(np.float32)
